# revision 15
# baseline (speedup 1.0000x reference)
"""BiAttention similarity kernel for Trainium2, 8-core data-parallel over batch.

Computes, per batch b:
    s0 = c @ c_weight                  # [L, 1]
    s1 = (c @ q_weight)^T              # [1, L]
    s2 = (c * cq_weight) @ q^T         # [L, L]
    s  = s0 + s1 + s2 + bias           # [L, L]

Shapes (hardcoded): B=8, L=2048, D=256, fp32 in/out.

Distribution strategy: data-parallel over batch, one batch per core.

Algebraic folding: the device computes TRANSPOSED tiles
    sT[j, i] = sum_k qaugT[k, j] * cT[k, i] + (s1[j] + bias)
with qaug = q * cq_weight + c_weight^T prepared on host. The +c_weight
augmentation contracts against cT to produce exactly s0[i] broadcast over j,
so the rank-2 (s0 + s1 + bias) field costs zero extra PE passes:
  - s0 rides inside the main GEMM (operand augmentation)
  - s1[j] + bias is per-partition in the transposed layout and is folded
    into the PSUM->SBUF copy as the bias of an ACT Identity / DVE
    tensor_scalar add.
Per [128, 512] output tile the device does only 2 matmuls (K=128 each)
plus one copy-with-bias. The host transposes each core's sT result back.

The device emits sT in fp16 (the copy-with-bias downcasts from fp32 PSUM)
and the host upcasts to fp32: output quantization adds ~3e-4 relative
error but halves the dominant HBM write traffic (16.8 -> 8.4 MB per core,
vs the ~360 GB/s per-core HBM share that both DMA queues together were
already saturating).

Layout/engine plan:
  - inputs: first qaugT 128-column chunk + cT k=0 quarters on the SP ring,
    the other on the DVE ring, qaugT remainders on ACT/DVE, so the PE can
    start after ~0.6 MB of loads and never starves
  - per row-chunk: 8 matmuls (weight-stationary: 2 LDWEIGHTS), 4
    copy-with-bias ops alternating ScalarE/VectorE, one 512 KiB output DMA
  - output DMAs alternate SP and Pool rings
"""

import numpy as np
from contextlib import ExitStack

import concourse.bass as bass
import concourse.tile as tile
from concourse import bacc, mybir
from concourse.bass_utils import run_bass_kernel_spmd

F32 = mybir.dt.float32
F16 = mybir.dt.float16

B = 8
L = 2048
D = 256
NK = D // 128          # 2 contraction chunks of 128
NT = L // 128          # 16 row chunks (j, on partitions; transposed layout)
TI = 512               # moving free dim; matmul output must fit one PSUM bank
NI = L // TI

# set by test harness to request an NTFF trace; results stashed in LAST_RESULTS
TRACE = False
LAST_RESULTS = None

_NC_CACHE = None


def build_body(ctx: ExitStack, tc: tile.TileContext, aps: dict):
    nc = tc.nc
    ct_d, qt_d, s1_d, s_d = aps["ct"], aps["qaugt"], aps["s1c"], aps["s"]

    consts = ctx.enter_context(tc.tile_pool(name="consts", bufs=1))
    psum = ctx.enter_context(tc.tile_pool(name="psum", bufs=4, space="PSUM"))
    outp = ctx.enter_context(tc.tile_pool(name="outp", bufs=16))

    # s1[j] + bias, laid out [128, NT]: column t holds the per-partition
    # bias vector for row-chunk t
    s1sb = consts.tile([128, NT], F32)

    cT = [consts.tile([128, L], F16, tag=f"cT{k}", name=f"cT{k}")
          for k in range(NK)]
    qT = [consts.tile([128, L], F16, tag=f"qT{k}", name=f"qT{k}")
          for k in range(NK)]

    # PE clock warmup: the Tensor engine DVFS-ramps to full speed only after
    # ~3us of continuous execution. Zero-matmuls (on memset tiles, into a
    # scratch PSUM pair never read back) keep the PE busy through the input
    # load window so the real stream starts at full clock.
    dw = consts.tile([128, 130], F16, tag="dw", name="dw")
    nc.gpsimd.memset(dw[:], 0.0)
    dscr = psum.tile([128, 2 * TI], F32, tag="main", name="dscr")
    for _ in range(30):
        nc.tensor.matmul(dscr[0:1, 0:128], dw[:, 0:1], dw[:, 2:130],
                         start=True, stop=True)

    # First-chunk gate loads spread over all three DMA rings in
    # earliest-deadline order (ring cadence ~0.45us/quarter, first item
    # ready ~10us); the remaining qaugT columns stream on the Pool/SWDGE
    # ring in pieces sized to stay ahead of the row-chunk consuming them.
    Q = [slice(q * 512, (q + 1) * 512) for q in range(4)]
    nc.sync.dma_start(qT[0][:, 0:128], qt_d[0:128, 0:128])
    nc.scalar.dma_start(cT[0][:, Q[0]], ct_d[0:128, Q[0]])
    nc.gpsimd.dma_start(cT[0][:, Q[1]], ct_d[0:128, Q[1]])
    nc.sync.dma_start(cT[0][:, Q[2]], ct_d[0:128, Q[2]])
    nc.scalar.dma_start(cT[0][:, Q[3]], ct_d[0:128, Q[3]])
    nc.gpsimd.dma_start(qT[1][:, 0:128], qt_d[128:256, 0:128])
    nc.sync.dma_start(cT[1][:, Q[0]], ct_d[128:256, Q[0]])
    nc.scalar.dma_start(cT[1][:, Q[1]], ct_d[128:256, Q[1]])
    nc.gpsimd.dma_start(cT[1][:, Q[2]], ct_d[128:256, Q[2]])
    nc.sync.dma_start(cT[1][:, Q[3]], ct_d[128:256, Q[3]])
    nc.scalar.dma_start(qT[0][:, 128:512], qt_d[0:128, 128:512])
    nc.gpsimd.dma_start(qT[1][:, 128:512], qt_d[128:256, 128:512])
    nc.sync.dma_start(s1sb[:], s1_d[:, :])
    for lo, hi in ((512, 1024), (1024, 2048)):
        nc.gpsimd.dma_start(qT[0][:, lo:hi], qt_d[0:128, lo:hi])
        nc.gpsimd.dma_start(qT[1][:, lo:hi], qt_d[128:256, lo:hi])

    # ---- main loop: 16 row-chunks x 4 moving tiles ----------------------
    # psum tiles span 2 banks; matmuls land in 512-col bank slices, the
    # copy-with-bias reads 1024 cols in one op (DVE low half, ACT high half)
    HN = L // 2
    for t in range(NT):
        tsl = slice(t * 128, (t + 1) * 128)
        out_sb = outp.tile([128, L], F16, tag="out", name="out_sb")
        psA = psum.tile([128, 2 * TI], F32, tag="main", name="psA")
        psB = psum.tile([128, 2 * TI], F32, tag="main", name="psB")
        pss = [psA[:, 0:TI], psA[:, TI:2 * TI],
               psB[:, 0:TI], psB[:, TI:2 * TI]]
        # weight-stationary: hold each qaugT chunk across all NI tiles
        for ii in range(NI):
            nc.tensor.matmul(pss[ii], qT[0][:, tsl],
                             cT[0][:, ii * TI:(ii + 1) * TI],
                             start=True, stop=False)
        for ii in range(NI):
            nc.tensor.matmul(pss[ii], qT[1][:, tsl],
                             cT[1][:, ii * TI:(ii + 1) * TI],
                             start=False, stop=True)
        # PSUM->SBUF copy fused with the +(s1[j]+bias) per-partition add.
        # The last chunks drain on the HWDGE rings (SP + ACT, both idle by
        # then) so the SWDGE ring is long done before the epilogue flush,
        # and the final chunk goes at tile granularity to shorten the tail.
        if t < NT - 1:
            nc.vector.tensor_scalar_add(out_sb[:, 0:HN], psA[:],
                                        s1sb[:, t:t + 1])
            nc.scalar.add(out_sb[:, HN:L], psB[:], s1sb[:, t:t + 1])
            nc.sync.dma_start(s_d[tsl, 0:HN], out_sb[:, 0:HN])
            nc.gpsimd.dma_start(s_d[tsl, HN:L], out_sb[:, HN:L])
        else:
            for ii in range(NI):
                isl = slice(ii * TI, (ii + 1) * TI)
                if ii % 2 == 0:
                    nc.vector.tensor_scalar_add(out_sb[:, isl], pss[ii],
                                                s1sb[:, t:t + 1])
                else:
                    nc.scalar.add(out_sb[:, isl], pss[ii], s1sb[:, t:t + 1])
                (nc.scalar if ii % 2 == 0 else nc.sync).dma_start(
                    s_d[tsl, isl], out_sb[:, isl])


def build_nc():
    nc = bacc.Bacc("TRN2", target_bir_lowering=False, debug=False)
    aps = {
        "ct": nc.dram_tensor("ct", [D, L], F16, kind="ExternalInput").ap(),
        "qaugt": nc.dram_tensor("qaugt", [D, L], F16,
                                kind="ExternalInput").ap(),
        "s1c": nc.dram_tensor("s1c", [128, NT], F32,
                              kind="ExternalInput").ap(),
        "s": nc.dram_tensor("s", [L, L], F16, kind="ExternalOutput").ap(),
    }
    with tile.TileContext(nc) as tc:
        with ExitStack() as ctx:
            build_body(ctx, tc, aps)
    nc.compile()
    return nc


def get_nc():
    global _NC_CACHE
    if _NC_CACHE is None:
        _NC_CACHE = build_nc()
    return _NC_CACHE


def kernel(c, q, c_weight, q_weight, cq_weight, bias):
    global LAST_RESULTS
    nc = get_nc()
    c = np.asarray(c, dtype=np.float32)
    q = np.asarray(q, dtype=np.float32)
    cw = np.asarray(c_weight, dtype=np.float32)[:, 0]       # [D]
    qw = np.asarray(q_weight, dtype=np.float32)[:, 0]       # [D]
    cqw = np.asarray(cq_weight, dtype=np.float32)[0, 0]     # [D]
    bias = float(np.asarray(bias, dtype=np.float32)[0])
    in_maps = []
    for b in range(B):
        qaug = q[b] * cqw + cw                              # [L, D]
        s1 = c[b] @ qw + bias                               # [L]
        in_maps.append({
            "ct": np.ascontiguousarray(c[b].T).astype(np.float16),
            "qaugt": np.ascontiguousarray(qaug.T).astype(np.float16),
            "s1c": np.ascontiguousarray(s1.reshape(NT, 128).T),
        })
    res = run_bass_kernel_spmd(nc, in_maps, core_ids=list(range(B)), trace=TRACE)
    LAST_RESULTS = res
    return np.stack([res.results[b]["s"].T.astype(np.float32)
                     for b in range(B)], axis=0)


# revision 17
# speedup vs baseline: 1.0200x; 1.0200x over previous
"""BiAttention similarity kernel for Trainium2, 8-core data-parallel over batch.

Computes, per batch b:
    s0 = c @ c_weight                  # [L, 1]
    s1 = (c @ q_weight)^T              # [1, L]
    s2 = (c * cq_weight) @ q^T         # [L, L]
    s  = s0 + s1 + s2 + bias           # [L, L]

Shapes (hardcoded): B=8, L=2048, D=256, fp32 in/out.

Distribution strategy: data-parallel over batch, one batch per core.

Algebraic folding: the device computes TRANSPOSED tiles
    sT[j, i] = sum_k qaugT[k, j] * cT[k, i] + (s1[j] + bias)
with qaug = q * cq_weight + c_weight^T prepared on host. The +c_weight
augmentation contracts against cT to produce exactly s0[i] broadcast over j,
so the rank-2 (s0 + s1 + bias) field costs zero extra PE passes:
  - s0 rides inside the main GEMM (operand augmentation)
  - s1[j] + bias is per-partition in the transposed layout and is folded
    into the PSUM->SBUF copy as the bias of an ACT Identity / DVE
    tensor_scalar add.
Per [128, 512] output tile the device does only 2 matmuls (K=128 each)
plus one copy-with-bias. The host transposes each core's sT result back.

The device emits sT in fp16 (the copy-with-bias downcasts from fp32 PSUM)
and the host upcasts to fp32: output quantization adds ~3e-4 relative
error but halves the dominant HBM write traffic (16.8 -> 8.4 MB per core,
vs the ~360 GB/s per-core HBM share that both DMA queues together were
already saturating).

Layout/engine plan:
  - inputs: first qaugT 128-column chunk + cT k=0 quarters on the SP ring,
    the other on the DVE ring, qaugT remainders on ACT/DVE, so the PE can
    start after ~0.6 MB of loads and never starves
  - per row-chunk: 8 matmuls (weight-stationary: 2 LDWEIGHTS), 4
    copy-with-bias ops alternating ScalarE/VectorE, one 512 KiB output DMA
  - output DMAs alternate SP and Pool rings
"""

import numpy as np
from contextlib import ExitStack

import concourse.bass as bass
import concourse.tile as tile
from concourse import bacc, mybir
from concourse.bass_utils import run_bass_kernel_spmd

F32 = mybir.dt.float32
F16 = mybir.dt.float16

B = 8
L = 2048
D = 256
NK = D // 128          # 2 contraction chunks of 128
NT = L // 128          # 16 row chunks (j, on partitions; transposed layout)
TI = 512               # moving free dim; matmul output must fit one PSUM bank
NI = L // TI

# set by test harness to request an NTFF trace; results stashed in LAST_RESULTS
TRACE = False
LAST_RESULTS = None

_NC_CACHE = None


def build_body(ctx: ExitStack, tc: tile.TileContext, aps: dict):
    nc = tc.nc
    ct_d, qt_d, s1_d, s_d = aps["ct"], aps["qaugt"], aps["s1c"], aps["s"]

    consts = ctx.enter_context(tc.tile_pool(name="consts", bufs=1))
    psum = ctx.enter_context(tc.tile_pool(name="psum", bufs=4, space="PSUM"))
    outp = ctx.enter_context(tc.tile_pool(name="outp", bufs=16))

    # s1[j] + bias, laid out [128, NT]: column t holds the per-partition
    # bias vector for row-chunk t
    s1sb = consts.tile([128, NT], F32)

    cT = [consts.tile([128, L], F16, tag=f"cT{k}", name=f"cT{k}")
          for k in range(NK)]
    qT = [consts.tile([128, L], F16, tag=f"qT{k}", name=f"qT{k}")
          for k in range(NK)]

    # PE clock warmup: the Tensor engine DVFS-ramps to full speed only after
    # ~3us of continuous execution. Zero-matmuls (on memset tiles, into a
    # scratch PSUM pair never read back) keep the PE busy through the input
    # load window so the real stream starts at full clock.
    dw = consts.tile([128, 130], F16, tag="dw", name="dw")
    nc.gpsimd.memset(dw[:], 0.0)
    dscr = psum.tile([128, 2 * TI], F32, tag="main", name="dscr")
    for _ in range(30):
        nc.tensor.matmul(dscr[0:1, 0:128], dw[:, 0:1], dw[:, 2:130],
                         start=True, stop=True)

    # First-chunk gate loads spread over all three DMA rings in
    # earliest-deadline order (ring cadence ~0.45us/quarter, first item
    # ready ~10us); the remaining qaugT columns stream on the Pool/SWDGE
    # ring in pieces sized to stay ahead of the row-chunk consuming them.
    Q = [slice(q * 512, (q + 1) * 512) for q in range(4)]
    nc.sync.dma_start(qT[0][:, 0:128], qt_d[0:128, 0:128])
    nc.scalar.dma_start(cT[0][:, Q[0]], ct_d[0:128, Q[0]])
    nc.gpsimd.dma_start(cT[0][:, Q[1]], ct_d[0:128, Q[1]])
    nc.sync.dma_start(cT[0][:, Q[2]], ct_d[0:128, Q[2]])
    nc.scalar.dma_start(cT[0][:, Q[3]], ct_d[0:128, Q[3]])
    nc.gpsimd.dma_start(qT[1][:, 0:128], qt_d[128:256, 0:128])
    nc.sync.dma_start(cT[1][:, Q[0]], ct_d[128:256, Q[0]])
    nc.scalar.dma_start(cT[1][:, Q[1]], ct_d[128:256, Q[1]])
    nc.gpsimd.dma_start(cT[1][:, Q[2]], ct_d[128:256, Q[2]])
    nc.sync.dma_start(cT[1][:, Q[3]], ct_d[128:256, Q[3]])
    nc.scalar.dma_start(qT[0][:, 128:256], qt_d[0:128, 128:256])
    nc.gpsimd.dma_start(qT[1][:, 128:256], qt_d[128:256, 128:256])
    nc.sync.dma_start(s1sb[:], s1_d[:, :])
    nc.scalar.dma_start(qT[0][:, 256:1024], qt_d[0:128, 256:1024])
    nc.gpsimd.dma_start(qT[1][:, 256:1024], qt_d[128:256, 256:1024])
    nc.gpsimd.dma_start(qT[0][:, 1024:2048], qt_d[0:128, 1024:2048])
    nc.gpsimd.dma_start(qT[1][:, 1024:2048], qt_d[128:256, 1024:2048])

    # ---- main loop: 16 row-chunks x 4 moving tiles ----------------------
    # psum tiles span 2 banks; matmuls land in 512-col bank slices, the
    # copy-with-bias reads 1024 cols in one op (DVE low half, ACT high half)
    HN = L // 2
    for t in range(NT):
        tsl = slice(t * 128, (t + 1) * 128)
        out_sb = outp.tile([128, L], F16, tag="out", name="out_sb")
        psA = psum.tile([128, 2 * TI], F32, tag="main", name="psA")
        psB = psum.tile([128, 2 * TI], F32, tag="main", name="psB")
        pss = [psA[:, 0:TI], psA[:, TI:2 * TI],
               psB[:, 0:TI], psB[:, TI:2 * TI]]
        # weight-stationary: hold each qaugT chunk across all NI tiles
        for ii in range(NI):
            nc.tensor.matmul(pss[ii], qT[0][:, tsl],
                             cT[0][:, ii * TI:(ii + 1) * TI],
                             start=True, stop=False)
        for ii in range(NI):
            nc.tensor.matmul(pss[ii], qT[1][:, tsl],
                             cT[1][:, ii * TI:(ii + 1) * TI],
                             start=False, stop=True)
        # PSUM->SBUF copy fused with the +(s1[j]+bias) per-partition add.
        # The last chunks drain on the HWDGE rings (SP + ACT, both idle by
        # then) so the SWDGE ring is long done before the epilogue flush,
        # and the final chunk goes at tile granularity to shorten the tail.
        if t < NT - 1:
            nc.vector.tensor_scalar_add(out_sb[:, 0:HN], psA[:],
                                        s1sb[:, t:t + 1])
            nc.scalar.add(out_sb[:, HN:L], psB[:], s1sb[:, t:t + 1])
            nc.sync.dma_start(s_d[tsl, 0:HN], out_sb[:, 0:HN])
            nc.gpsimd.dma_start(s_d[tsl, HN:L], out_sb[:, HN:L])
        else:
            for ii in range(NI):
                isl = slice(ii * TI, (ii + 1) * TI)
                if ii % 2 == 0:
                    nc.vector.tensor_scalar_add(out_sb[:, isl], pss[ii],
                                                s1sb[:, t:t + 1])
                else:
                    nc.scalar.add(out_sb[:, isl], pss[ii], s1sb[:, t:t + 1])
                # DVE-copied tiles drain via SP so the ACT queue never
                # blocks on a cross-engine semaphore
                (nc.sync if ii % 2 == 0 else nc.scalar).dma_start(
                    s_d[tsl, isl], out_sb[:, isl])


def build_nc():
    nc = bacc.Bacc("TRN2", target_bir_lowering=False, debug=False)
    aps = {
        "ct": nc.dram_tensor("ct", [D, L], F16, kind="ExternalInput").ap(),
        "qaugt": nc.dram_tensor("qaugt", [D, L], F16,
                                kind="ExternalInput").ap(),
        "s1c": nc.dram_tensor("s1c", [128, NT], F32,
                              kind="ExternalInput").ap(),
        "s": nc.dram_tensor("s", [L, L], F16, kind="ExternalOutput").ap(),
    }
    with tile.TileContext(nc) as tc:
        with ExitStack() as ctx:
            build_body(ctx, tc, aps)
    nc.compile()
    return nc


def get_nc():
    global _NC_CACHE
    if _NC_CACHE is None:
        _NC_CACHE = build_nc()
    return _NC_CACHE


def kernel(c, q, c_weight, q_weight, cq_weight, bias):
    global LAST_RESULTS
    nc = get_nc()
    c = np.asarray(c, dtype=np.float32)
    q = np.asarray(q, dtype=np.float32)
    cw = np.asarray(c_weight, dtype=np.float32)[:, 0]       # [D]
    qw = np.asarray(q_weight, dtype=np.float32)[:, 0]       # [D]
    cqw = np.asarray(cq_weight, dtype=np.float32)[0, 0]     # [D]
    bias = float(np.asarray(bias, dtype=np.float32)[0])
    in_maps = []
    for b in range(B):
        qaug = q[b] * cqw + cw                              # [L, D]
        s1 = c[b] @ qw + bias                               # [L]
        in_maps.append({
            "ct": np.ascontiguousarray(c[b].T).astype(np.float16),
            "qaugt": np.ascontiguousarray(qaug.T).astype(np.float16),
            "s1c": np.ascontiguousarray(s1.reshape(NT, 128).T),
        })
    res = run_bass_kernel_spmd(nc, in_maps, core_ids=list(range(B)), trace=TRACE)
    LAST_RESULTS = res
    return np.stack([res.results[b]["s"].T.astype(np.float32)
                     for b in range(B)], axis=0)


# revision 19
# speedup vs baseline: 1.0402x; 1.0198x over previous
"""BiAttention similarity kernel for Trainium2, 8-core data-parallel over batch.

Computes, per batch b:
    s0 = c @ c_weight                  # [L, 1]
    s1 = (c @ q_weight)^T              # [1, L]
    s2 = (c * cq_weight) @ q^T         # [L, L]
    s  = s0 + s1 + s2 + bias           # [L, L]

Shapes (hardcoded): B=8, L=2048, D=256, fp32 in/out.

Distribution strategy: data-parallel over batch, one batch per core.

Algebraic folding: the device computes TRANSPOSED tiles
    sT[j, i] = sum_k qaugT[k, j] * cT[k, i] + (s1[j] + bias)
with qaug = q * cq_weight + c_weight^T prepared on host. The +c_weight
augmentation contracts against cT to produce exactly s0[i] broadcast over j,
so the rank-2 (s0 + s1 + bias) field costs zero extra PE passes:
  - s0 rides inside the main GEMM (operand augmentation)
  - s1[j] + bias is per-partition in the transposed layout and is folded
    into the PSUM->SBUF copy as the bias of an ACT Identity / DVE
    tensor_scalar add.
Per [128, 512] output tile the device does only 2 matmuls (K=128 each)
plus one copy-with-bias. The host transposes each core's sT result back.

The device emits sT in fp16 (the copy-with-bias downcasts from fp32 PSUM)
and the host upcasts to fp32: output quantization adds ~3e-4 relative
error but halves the dominant HBM write traffic (16.8 -> 8.4 MB per core,
vs the ~360 GB/s per-core HBM share that both DMA queues together were
already saturating).

Layout/engine plan:
  - inputs: first qaugT 128-column chunk + cT k=0 quarters on the SP ring,
    the other on the DVE ring, qaugT remainders on ACT/DVE, so the PE can
    start after ~0.6 MB of loads and never starves
  - per row-chunk: 8 matmuls (weight-stationary: 2 LDWEIGHTS), 4
    copy-with-bias ops alternating ScalarE/VectorE, one 512 KiB output DMA
  - output DMAs alternate SP and Pool rings
"""

import numpy as np
from contextlib import ExitStack

import concourse.bass as bass
import concourse.tile as tile
from concourse import bacc, mybir
from concourse.bass_utils import run_bass_kernel_spmd

F32 = mybir.dt.float32
F16 = mybir.dt.float16

B = 8
L = 2048
D = 256
NK = D // 128          # 2 contraction chunks of 128
NT = L // 128          # 16 row chunks (j, on partitions; transposed layout)
TI = 512               # moving free dim; matmul output must fit one PSUM bank
NI = L // TI

# set by test harness to request an NTFF trace; results stashed in LAST_RESULTS
TRACE = False
LAST_RESULTS = None

_NC_CACHE = None


def build_body(ctx: ExitStack, tc: tile.TileContext, aps: dict):
    nc = tc.nc
    ct_d, qt_d, s1_d, s_d = aps["ct"], aps["qaugt"], aps["s1c"], aps["s"]

    consts = ctx.enter_context(tc.tile_pool(name="consts", bufs=1))
    psum = ctx.enter_context(tc.tile_pool(name="psum", bufs=4, space="PSUM"))
    outp = ctx.enter_context(tc.tile_pool(name="outp", bufs=16))

    # s1[j] + bias, laid out [128, NT]: column t holds the per-partition
    # bias vector for row-chunk t
    s1sb = consts.tile([128, NT], F32)

    cT = [consts.tile([128, L], F16, tag=f"cT{k}", name=f"cT{k}")
          for k in range(NK)]
    qT = [consts.tile([128, L], F16, tag=f"qT{k}", name=f"qT{k}")
          for k in range(NK)]

    # PE clock warmup: the Tensor engine DVFS-ramps to full speed only after
    # ~3us of continuous execution. Zero-matmuls (on memset tiles, into a
    # scratch PSUM pair never read back) keep the PE busy through the input
    # load window so the real stream starts at full clock.
    dw = consts.tile([128, 130], F16, tag="dw", name="dw")
    nc.gpsimd.memset(dw[:], 0.0)
    dscr = psum.tile([128, 2 * TI], F32, tag="main", name="dscr")
    for _ in range(32):
        nc.tensor.matmul(dscr[0:1, 0:128], dw[:, 0:1], dw[:, 2:130],
                         start=True, stop=True)

    # First-chunk gate loads spread over all three DMA rings in
    # earliest-deadline order (ring cadence ~0.45us/quarter, first item
    # ready ~10us); the remaining qaugT columns stream on the Pool/SWDGE
    # ring in pieces sized to stay ahead of the row-chunk consuming them.
    Q = [slice(q * 512, (q + 1) * 512) for q in range(4)]
    nc.sync.dma_start(qT[0][:, 0:128], qt_d[0:128, 0:128])
    nc.scalar.dma_start(cT[0][:, Q[0]], ct_d[0:128, Q[0]])
    nc.gpsimd.dma_start(cT[0][:, Q[1]], ct_d[0:128, Q[1]])
    nc.sync.dma_start(cT[0][:, Q[2]], ct_d[0:128, Q[2]])
    nc.scalar.dma_start(cT[0][:, Q[3]], ct_d[0:128, Q[3]])
    nc.gpsimd.dma_start(qT[1][:, 0:128], qt_d[128:256, 0:128])
    nc.sync.dma_start(cT[1][:, Q[0]], ct_d[128:256, Q[0]])
    nc.scalar.dma_start(cT[1][:, Q[1]], ct_d[128:256, Q[1]])
    nc.gpsimd.dma_start(cT[1][:, Q[2]], ct_d[128:256, Q[2]])
    nc.sync.dma_start(cT[1][:, Q[3]], ct_d[128:256, Q[3]])
    nc.scalar.dma_start(qT[0][:, 128:256], qt_d[0:128, 128:256])
    nc.sync.dma_start(qT[1][:, 128:256], qt_d[128:256, 128:256])
    nc.gpsimd.dma_start(s1sb[:], s1_d[:, :])
    nc.scalar.dma_start(qT[0][:, 256:1024], qt_d[0:128, 256:1024])
    nc.gpsimd.dma_start(qT[1][:, 256:1024], qt_d[128:256, 256:1024])
    nc.gpsimd.dma_start(qT[0][:, 1024:2048], qt_d[0:128, 1024:2048])
    nc.gpsimd.dma_start(qT[1][:, 1024:2048], qt_d[128:256, 1024:2048])

    # ---- main loop: 16 row-chunks x 4 moving tiles ----------------------
    # psum tiles span 2 banks; matmuls land in 512-col bank slices, the
    # copy-with-bias reads 1024 cols in one op (DVE low half, ACT high half)
    HN = L // 2
    for t in range(NT):
        tsl = slice(t * 128, (t + 1) * 128)
        out_sb = outp.tile([128, L], F16, tag="out", name="out_sb")
        psA = psum.tile([128, 2 * TI], F32, tag="main", name="psA")
        psB = psum.tile([128, 2 * TI], F32, tag="main", name="psB")
        pss = [psA[:, 0:TI], psA[:, TI:2 * TI],
               psB[:, 0:TI], psB[:, TI:2 * TI]]
        # weight-stationary: hold each qaugT chunk across all NI tiles
        for ii in range(NI):
            nc.tensor.matmul(pss[ii], qT[0][:, tsl],
                             cT[0][:, ii * TI:(ii + 1) * TI],
                             start=True, stop=False)
        for ii in range(NI):
            nc.tensor.matmul(pss[ii], qT[1][:, tsl],
                             cT[1][:, ii * TI:(ii + 1) * TI],
                             start=False, stop=True)
        # PSUM->SBUF copy fused with the +(s1[j]+bias) per-partition add.
        # The last chunks drain on the HWDGE rings (SP + ACT, both idle by
        # then) so the SWDGE ring is long done before the epilogue flush,
        # and the final chunk goes at tile granularity to shorten the tail.
        if t < NT - 1:
            nc.vector.tensor_scalar_add(out_sb[:, 0:HN], psA[:],
                                        s1sb[:, t:t + 1])
            nc.scalar.add(out_sb[:, HN:L], psB[:], s1sb[:, t:t + 1])
            nc.sync.dma_start(s_d[tsl, 0:HN], out_sb[:, 0:HN])
            nc.gpsimd.dma_start(s_d[tsl, HN:L], out_sb[:, HN:L])
        else:
            for ii in range(NI):
                isl = slice(ii * TI, (ii + 1) * TI)
                if ii % 2 == 0:
                    nc.vector.tensor_scalar_add(out_sb[:, isl], pss[ii],
                                                s1sb[:, t:t + 1])
                else:
                    nc.scalar.add(out_sb[:, isl], pss[ii], s1sb[:, t:t + 1])
                # DVE-copied tiles drain via SP so the ACT queue never
                # blocks on a cross-engine semaphore
                (nc.sync if ii % 2 == 0 else nc.scalar).dma_start(
                    s_d[tsl, isl], out_sb[:, isl])


def build_nc():
    nc = bacc.Bacc("TRN2", target_bir_lowering=False, debug=False)
    aps = {
        "ct": nc.dram_tensor("ct", [D, L], F16, kind="ExternalInput").ap(),
        "qaugt": nc.dram_tensor("qaugt", [D, L], F16,
                                kind="ExternalInput").ap(),
        "s1c": nc.dram_tensor("s1c", [128, NT], F32,
                              kind="ExternalInput").ap(),
        "s": nc.dram_tensor("s", [L, L], F16, kind="ExternalOutput").ap(),
    }
    with tile.TileContext(nc) as tc:
        with ExitStack() as ctx:
            build_body(ctx, tc, aps)
    nc.compile()
    return nc


def get_nc():
    global _NC_CACHE
    if _NC_CACHE is None:
        _NC_CACHE = build_nc()
    return _NC_CACHE


def kernel(c, q, c_weight, q_weight, cq_weight, bias):
    global LAST_RESULTS
    nc = get_nc()
    c = np.asarray(c, dtype=np.float32)
    q = np.asarray(q, dtype=np.float32)
    cw = np.asarray(c_weight, dtype=np.float32)[:, 0]       # [D]
    qw = np.asarray(q_weight, dtype=np.float32)[:, 0]       # [D]
    cqw = np.asarray(cq_weight, dtype=np.float32)[0, 0]     # [D]
    bias = float(np.asarray(bias, dtype=np.float32)[0])
    in_maps = []
    for b in range(B):
        qaug = q[b] * cqw + cw                              # [L, D]
        s1 = c[b] @ qw + bias                               # [L]
        in_maps.append({
            "ct": np.ascontiguousarray(c[b].T).astype(np.float16),
            "qaugt": np.ascontiguousarray(qaug.T).astype(np.float16),
            "s1c": np.ascontiguousarray(s1.reshape(NT, 128).T),
        })
    res = run_bass_kernel_spmd(nc, in_maps, core_ids=list(range(B)), trace=TRACE)
    LAST_RESULTS = res
    return np.stack([res.results[b]["s"].T.astype(np.float32)
                     for b in range(B)], axis=0)
